# revision 45
# baseline (speedup 1.0000x reference)
"""CRF NLL kernel for Trainium2 (8 NeuronCores, data-parallel over batch).

Self-contained: hardcodes shapes BS=8192, T=512, K=5.

Math: the 5-state CRF collapses to 3 live states {B,I,O} (START row and
STOP column of exp(transitions) are exactly 0).  The forward algorithm
runs in exp space.  Two time steps are fused into one "superstep":

    a_{2s+2} = W2_s @ a_{2s},   W2_s = D_{2s+2} (E3 D_{2s+1} E3)

The 3x3 W2_s matrices are built in bulk on the GPSIMD engine (E3 has 3
structurally-zero entries from the masked transitions, so E D E needs
only 12 nonzero multiply-accumulate terms, expressed as 7 regular-AP
ops), while the vector engine runs the serial chain at 2 ops per
superstep (multiply + segmented reduce).  Max-normalization every 6
supersteps; norm factors are log'd in bulk at the end.

Gold path score: per-partition accumulators via compare ops
(trans: 9-bin counts of idx=3*tag_t+tag_{t-1} dotted with the 3x3
table; emit: (tag==k)*f_k sweeps; plus START/STOP boundary terms).

Data parallel: batch 8192 -> 8 cores x 1024; per core 1024 = 8 groups
x 128 partitions.  Per-core partials summed on the host.
"""

import numpy as np
from contextlib import ExitStack

BS, T, K = 8192, 512, 5
NCORES = 8
BSH = BS // NCORES      # 1024 batch per core
G = BSH // 128          # 8 groups
START, STOP = 3, 4
S = (T - 1) // 2        # 255 supersteps covering t=1..510; t=511 leftover
QS = 9                  # supersteps between max-normalizations
NE = len([s for s in range(S) if s % QS == QS - 1])

# time chunks aligned to superstep boundaries (odd width => superstep
# pairs (t_odd=2s+1, t_even=2s+2) never straddle a boundary).  First
# chunks are small so the DMA->exp->W2-build->serial pipeline fills fast.
CH_TW = [33, 32, 64, 64, 64, 64, 64, 64, 63]
NCH = len(CH_TW)
CH_T0 = [sum(CH_TW[:c]) for c in range(NCH)]
assert sum(CH_TW) == T
# supersteps s with both 2s+1 and 2s+2 in [t0, t0+tw)
CH_S0 = [CH_T0[c] // 2 for c in range(NCH)]
CH_SN = [
    (CH_T0[c] + CH_TW[c] - 3) // 2 - CH_S0[c] + 1 for c in range(NCH)
]
assert CH_S0[-1] + CH_SN[-1] == S
assert all(CH_S0[c] + CH_SN[c] == CH_S0[c + 1] for c in range(NCH - 1))

_cache = {}


def _build():
    import concourse.bacc as bacc
    import concourse.mybir as mybir
    from concourse.tile import TileContext
    from concourse.alu_op_type import AluOpType as op
    AF = mybir.ActivationFunctionType
    f32 = mybir.dt.float32
    bf16 = mybir.dt.bfloat16
    AX = mybir.AxisListType

    nc = bacc.Bacc(
        "TRN2", target_bir_lowering=False, debug=False, num_devices=NCORES
    )
    feat_p = nc.declare_dram_parameter("feature", [BSH, 3, T], bf16, isOutput=False)
    tags_p = nc.declare_dram_parameter("tags", [BSH, T], bf16, isOutput=False)
    cst_p = nc.declare_dram_parameter("consts", [128, 64], f32, isOutput=False)
    out_p = nc.declare_dram_parameter("out", [1, 4], f32, isOutput=True)

    featr = feat_p[:].rearrange("(g p) k t -> p k g t", p=128)
    tagsr = tags_p[:].rearrange("(g p) t -> p g t", p=128)

    # consts columns:
    #  0-8  E3[j,i] (j-major)         9-11 E5[0:3,START]
    # 12-14 E5[STOP,0:3]              15   ones
    # 16-24 tr3 flat                  25-27 tr[k,START]
    # 28-30 tr[STOP,k]
    # 32-37 M2 (k=2: j in {0,2} x i)  38-39 M1a (j=1, i in {0,1})
    # 40-41 M1b (j=2, i in {0,1})     42 M0a (j=1,i=2)   43 M0b (j=2,i=2)
    with TileContext(nc) as tc, ExitStack() as ctx:
        sb = ctx.enter_context(tc.tile_pool(name="sb", bufs=1))
        ps = ctx.enter_context(tc.tile_pool(name="ps", bufs=1, space="PSUM"))

        cst = sb.tile([128, 64], f32)
        tagsf = sb.tile([128, G, T], bf16)
        fbufs = []
        dodds = []
        devens = []
        w2s = []
        for c in range(NCH):
            tw = CH_TW[c]
            sn = CH_SN[c]
            fb = sb.tile([128, 3, G, tw], bf16, name=f"fb{c}")
            nc.sync.dma_start(
                out=fb[:], in_=featr[:, :, :, CH_T0[c] : CH_T0[c] + tw]
            )
            fbufs.append(fb)
            if c == 0:
                # small, needed by the first W2 build / init
                nc.sync.dma_start(out=cst[:], in_=cst_p[:])
            if c == 1:
                # tags (1 MB bf16): land early enough that the mask ops the
                # scheduler slots mid-serial never wait on them
                nc.sync.dma_start(out=tagsf[:], in_=tagsr[:])
            # exp of odd/even steps packed [p, s, g, 3] so (s,g) merges
            # into one AP dim for the 3D-limited STT build ops
            o_odd = 2 * CH_S0[c] + 1 - CH_T0[c]
            do = sb.tile([128, sn, G, 3], f32, name=f"do{c}")
            de = sb.tile([128, sn, G, 3], f32, name=f"de{c}")
            nc.scalar.activation(
                do[:],
                fb[:, :, :, o_odd : o_odd + 2 * sn - 1 : 2]
                .rearrange("p k g s -> p s g k"),
                AF.Exp,
            )
            nc.scalar.activation(
                de[:],
                fb[:, :, :, o_odd + 1 : o_odd + 2 * sn : 2]
                .rearrange("p k g s -> p s g k"),
                AF.Exp,
            )
            dodds.append(do)
            devens.append(de)
            w2 = sb.tile([128, sn, G, 9], bf16, name=f"w2{c}")
            w2s.append(w2)
        # d for the init step t=0 and the leftover step t=511
        dinit = sb.tile([128, G, 3], f32)
        dlast = sb.tile([128, G, 3], f32)
        nc.scalar.activation(
            dinit[:], fbufs[0][:, :, :, 0].rearrange("p k g -> p g k"), AF.Exp
        )
        nc.scalar.activation(
            dlast[:],
            fbufs[-1][:, :, :, T - 1 - CH_T0[-1]].rearrange("p k g -> p g k"),
            AF.Exp,
        )

        e3 = cst[:, 0:9].rearrange("p (j i) -> p j i", j=3)
        ecol = cst[:, 9:12]
        estop = cst[:, 12:15]
        ones = cst[:, 15:16]
        trf = cst[:, 16:25]

        # ---- bulk W2 build on GPSIMD ----
        # G2 = E3 D_odd E3 with E3 zeros at (0,0),(0,1),(1,2):
        #  k=2 -> entries {0,2}x{0,1,2}; k=1 -> {1,2}x{0,1}; k=0 -> {1,2}x{2}
        # overlaps: (2,0),(2,1) [k1+k2], (2,2) [k0+k2]
        # All APs 3D (walrus limit): (s,g) merged into one dim N = sn*G.
        tmpA = sb.tile([128, 32 * G, 2], f32)
        tmpB = sb.tile([128, 32 * G, 1], f32)
        def _build_w2(eng, w2F, doF, deF, n0, n1):
            N = n1 - n0
            w2f = w2F[:, n0:n1, :]
            dof = doF[:, n0:n1, :]
            def_ = deF[:, n0:n1, :]

            def dk(k):
                return dof[:, :, k : k + 1].broadcast_to((128, N, 3))

            def mc(c0, n):
                return (
                    cst[:, c0 : c0 + n].unsqueeze(1).broadcast_to((128, N, n))
                )

            # j=0 row <- d2 * M2[j0]
            eng.tensor_tensor(w2f[:, :, 0:3], dk(2), mc(32, 3), op.mult)
            # j=2 row <- d2 * M2[j2]
            eng.tensor_tensor(w2f[:, :, 6:9], dk(2), mc(35, 3), op.mult)
            # (1,0),(1,1) <- d1 * M1a
            eng.tensor_tensor(
                w2f[:, :, 3:5], dk(1)[:, :, 0:2], mc(38, 2), op.mult
            )
            # (2,0),(2,1) += d1 * M1b
            eng.tensor_tensor(
                tmpA[:, :N, :], dk(1)[:, :, 0:2], mc(40, 2), op.mult
            )
            eng.tensor_tensor(
                w2f[:, :, 6:8], w2f[:, :, 6:8], tmpA[:, :N, :], op.add
            )
            # (1,2) <- d0 * M0a
            eng.tensor_tensor(
                w2f[:, :, 5:6], dk(0)[:, :, 0:1], mc(42, 1), op.mult
            )
            # (2,2) += d0 * M0b
            eng.tensor_tensor(
                tmpB[:, :N, :], dk(0)[:, :, 0:1], mc(43, 1), op.mult
            )
            eng.tensor_tensor(
                w2f[:, :, 8:9], w2f[:, :, 8:9], tmpB[:, :N, :], op.add
            )
            # fold outer diag, one op per j-row: W2[j,:] *= d_even[j]
            for j in range(3):
                lb = eng.tensor_tensor(
                    w2f[:, :, 3 * j : 3 * j + 3],
                    w2f[:, :, 3 * j : 3 * j + 3],
                    def_[:, :, j : j + 1].broadcast_to((128, N, 3)),
                    op.mult,
                )
            return lb

        for c in range(NCH):
            do, de, w2 = dodds[c], devens[c], w2s[c]
            sn = CH_SN[c]
            w2F = w2[:].rearrange("p s g e -> p (s g) e")
            doF = do[:].rearrange("p s g k -> p (s g) k")
            deF = de[:].rearrange("p s g k -> p (s g) k")
            # chunk 0 built in two s-halves so the serial chain's first
            # supersteps get their W2 as early as possible
            if c == 0:
                half = (sn // 2) * G
                _build_w2(nc.gpsimd, w2F, doF, deF, 0, half)
                last_build = _build_w2(nc.gpsimd, w2F, doF, deF, half, sn * G)
            else:
                last_build = _build_w2(nc.gpsimd, w2F, doF, deF, 0, sn * G)

        # ---------------- serial chain ----------------
        a = sb.tile([128, G, 3], f32)
        tmp = sb.tile([128, G, 3, 3], f32)
        r = sb.tile([128, G], f32)
        mbuf = sb.tile([128, NE, G], f32)

        e3b = e3.unsqueeze(1).broadcast_to((128, G, 3, 3))
        ecolb = ecol.unsqueeze(1).broadcast_to((128, G, 3))
        estopb = estop.unsqueeze(1).broadcast_to((128, G, 3))

        # init: a = E5[:,START] * d_0
        nc.vector.tensor_tensor(a[:], dinit[:], ecolb, op.mult)

        s2c = []
        for c in range(NCH):
            s2c += [c] * CH_SN[c]
        ev = 0
        for s in range(S):
            c = s2c[s]
            ls = s - CH_S0[c]
            w2sl = (
                w2s[c][:, ls, :, :]
                .rearrange("p g (j i) -> p g j i", j=3)
            )
            ab = a[:].unsqueeze(2).broadcast_to((128, G, 3, 3))
            nc.vector.tensor_tensor(tmp[:], ab, w2sl, op.mult)
            nc.vector.tensor_reduce(a[:], tmp[:], axis=AX.X, op=op.add)
            if s % QS == QS - 1:
                nc.vector.tensor_reduce(mbuf[:, ev, :], a[:], axis=AX.X, op=op.max)
                nc.vector.reciprocal(r[:], mbuf[:, ev, :])
                rb = r[:].unsqueeze(2).broadcast_to((128, G, 3))
                nc.vector.tensor_tensor(a[:], a[:], rb, op.mult)
                ev += 1
        assert ev == NE

        # leftover step t = 511
        u = sb.tile([128, G, 3], f32)
        ab = a[:].unsqueeze(2).broadcast_to((128, G, 3, 3))
        nc.vector.tensor_tensor(tmp[:], ab, e3b, op.mult)
        nc.vector.tensor_reduce(u[:], tmp[:], axis=AX.X, op=op.add)
        nc.vector.tensor_tensor(a[:], u[:], dlast[:], op.mult)

        # terminal: fwd[p,g] = log(sum_j a[j]*estop[j]) + sum_e log(m[e])
        term = sb.tile([128, G], f32)
        flog = sb.tile([128, G], f32)
        last_serial = nc.vector.tensor_tensor(tmp[:, :, 0, :], a[:], estopb, op.mult)
        nc.vector.tensor_reduce(term[:], tmp[:, :, 0, :], axis=AX.X, op=op.add)
        nc.scalar.activation(flog[:], term[:], AF.Ln)

        mll = sb.tile([128, NE, G], f32)
        mred = sb.tile([128, G], f32)
        nc.scalar.activation(mll[:], mbuf[:], AF.Ln)
        mll_ge = mll[:].rearrange("p e g -> p g e")
        nc.vector.tensor_reduce(mred[:], mll_ge, axis=AX.X, op=op.add)

        ftot = sb.tile([128, G], f32)
        nc.vector.tensor_tensor(ftot[:], flog[:], mred[:], op.add)

        from concourse.bass import _add_dep_helper as add_dep

        # ---------------- gold path score ----------------
        idx = sb.tile([128, G, T - 1], bf16)
        junk = sb.tile([128, G * (T - 1)], bf16)
        junk2 = sb.tile([128, G, 65], bf16)
        cnts = sb.tile([128, 16], f32)
        eacc = sb.tile([128, NCH, 3], f32)

        # idx = 3*tag_t + tag_{t-1} on Pool (plain TTs; STT not Pool-legal)
        c3b = cst[:, 44:45].unsqueeze(1).broadcast_to((128, G, T - 1))
        ix0 = nc.gpsimd.tensor_tensor(
            idx[:], tagsf[:, :, 1:], c3b, op.mult
        )
        add_dep(ix0.ins, last_build.ins, reason="idx after W2 builds")
        idx_i = nc.gpsimd.tensor_tensor(
            idx[:], idx[:], tagsf[:, :, : T - 1], op.add
        )
        idxf = idx[:].rearrange("p g t -> p (g t)")
        for m in range(9):
            cnt_i = nc.vector.tensor_scalar(
                junk[:], idxf, float(m), None, op.is_equal, op.add,
                accum_out=cnts[:, m : m + 1],
            )
            add_dep(cnt_i.ins, last_serial.ins, reason="counts after serial chain")
        tmp9 = sb.tile([128, 9], f32)
        transp = sb.tile([128, 1], f32)
        nc.vector.tensor_tensor(tmp9[:, 0:8], cnts[:, 0:8], trf[:, 0:8], op.mult)
        nc.vector.tensor_reduce(transp[:], tmp9[:, 0:8], axis=AX.X, op=op.add)
        # + trf[8]*(cnt_8) with cnt_8 = (T-1)-sum(cnt_m): host stores
        # trf' = trf - trf8 in cols 16-23 and trf8*(T-1)*G... per-(p,g)
        # constant folded here per partition-row (G groups each):
        nc.vector.tensor_scalar(
            transp[:], transp[:], cst[:, 48:49], None, op.add
        )

        # emission gold: masks on DVE (is_equal not Pool-legal), the
        # mult/add accumulation on Pool, one final DVE reduce.
        maskb = sb.tile([128, 3, G, T], bf16)
        junk2b = sb.tile([128, G * T], bf16)
        prodb = sb.tile([128, G, T], bf16)
        for k in range(3):
            nc.vector.tensor_scalar(
                maskb[:, k], tagsf[:], float(k), None, op.is_equal
            )
            for c in range(NCH):
                tw = CH_TW[c]
                tsl = slice(CH_T0[c], CH_T0[c] + tw)
                msl = maskb[:, k, :, tsl]
                if k == 0:
                    p_i = nc.gpsimd.tensor_tensor(
                        prodb[:, :, tsl], msl, fbufs[c][:, 0, :, :], op.mult
                    )
                else:
                    p_i = nc.gpsimd.tensor_tensor(
                        junk2[:, :, :tw], msl, fbufs[c][:, k, :, :], op.mult
                    )
                    nc.gpsimd.tensor_tensor(
                        prodb[:, :, tsl], prodb[:, :, tsl], junk2[:, :, :tw],
                        op.add,
                    )
                add_dep(p_i.ins, last_build.ins, reason="emit after W2 builds")
        em_i = nc.vector.tensor_scalar(
            junk2b[:], prodb[:].rearrange("p g t -> p (g t)"), 0.0, None,
            op.add, op.add, accum_out=eacc[:, 0, 0:1],
        )
        add_dep(em_i.ins, last_serial.ins, reason="emit reduce after serial")

        # boundary: tr[tag_0, START] and tr[STOP, tag_{T-1}]
        bnd = sb.tile([128, 6, G], f32)
        for k in range(3):
            b_i = nc.vector.tensor_scalar(
                bnd[:, k, :], tagsf[:, :, 0], float(k), cst[:, 25 + k : 26 + k],
                op.is_equal, op.mult,
            )
            add_dep(b_i.ins, last_serial.ins, reason="gold tail after serial")
            b_i = nc.vector.tensor_scalar(
                bnd[:, 3 + k, :], tagsf[:, :, T - 1], float(k), cst[:, 28 + k : 29 + k],
                op.is_equal, op.mult,
            )
            add_dep(b_i.ins, last_serial.ins, reason="gold tail after serial")
        bred = sb.tile([128, G], f32)
        nc.vector.tensor_reduce(bred[:], bnd[:].rearrange("p s g -> p g s"),
                                axis=AX.X, op=op.add)

        # ---------------- combine ----------------
        nllg = sb.tile([128, G], f32)
        nc.vector.tensor_tensor(nllg[:], ftot[:], bred[:], op.subtract)
        red1 = sb.tile([128, 4], f32)
        nc.vector.tensor_reduce(red1[:, 0:1], nllg[:], axis=AX.X, op=op.add)
        nc.vector.tensor_copy(red1[:, 1:2], eacc[:, 0, 0:1])
        tot = sb.tile([128, 1], f32)
        nc.vector.tensor_tensor(tot[:], red1[:, 0:1], transp[:], op.subtract)
        nc.vector.tensor_tensor(tot[:], tot[:], red1[:, 1:2], op.subtract)

        acc = ps.tile([1, 1], f32)
        nc.tensor.matmul(acc[:], ones, tot[:], start=True, stop=True)
        osb = sb.tile([1, 4], f32)
        nc.vector.memset(osb[:], 0.0)
        nc.vector.tensor_copy(osb[:, 0:1], acc[:])
        nc.sync.dma_start(out=out_p[:], in_=osb[:])

    nc.compile()
    return nc


def _get_nc():
    if "nc" not in _cache:
        _cache["nc"] = _build()
    return _cache["nc"]


def _prep_inputs(feature, tags, transitions):
    f = np.asarray(feature, dtype=np.float32)
    tg = np.asarray(tags)
    tr = np.asarray(transitions, dtype=np.float32)

    E5 = np.exp(tr)
    E3 = E5[:3, :3]
    consts = np.zeros((128, 64), np.float32)
    consts[:, 0:9] = E3.reshape(-1)[None, :]
    consts[:, 9:12] = E5[:3, START][None, :]
    consts[:, 12:15] = E5[STOP, :3][None, :]
    consts[:, 15] = 1.0
    trf9 = tr[:3, :3].reshape(-1).astype(np.float64)
    consts[:, 16:24] = (trf9[:8] - trf9[8])[None, :].astype(np.float32)
    consts[:, 24] = 0.0
    consts[:, 48] = np.float32(G * trf9[8] * (T - 1))
    consts[:, 25:28] = tr[:3, START][None, :]
    consts[:, 28:31] = tr[STOP, :3][None, :]
    # M tables for the sparse E3 D E3 build
    consts[:, 32:38] = np.array(
        [E3[0, 2] * E3[2, 0], E3[0, 2] * E3[2, 1], E3[0, 2] * E3[2, 2],
         E3[2, 2] * E3[2, 0], E3[2, 2] * E3[2, 1], E3[2, 2] * E3[2, 2]],
        np.float32)[None, :]
    consts[:, 38:40] = np.array(
        [E3[1, 1] * E3[1, 0], E3[1, 1] * E3[1, 1]], np.float32)[None, :]
    consts[:, 40:42] = np.array(
        [E3[2, 1] * E3[1, 0], E3[2, 1] * E3[1, 1]], np.float32)[None, :]
    consts[:, 42] = E3[1, 0] * E3[0, 2]
    consts[:, 43] = E3[2, 0] * E3[0, 2]
    consts[:, 44] = 3.0
    consts[:, 45] = 0.0
    consts[:, 46] = 1.0
    consts[:, 47] = 2.0

    import ml_dtypes
    f3 = np.ascontiguousarray(f[:, :, :3].transpose(0, 2, 1)).astype(
        ml_dtypes.bfloat16
    )
    tgf = tg.astype(ml_dtypes.bfloat16)

    in_maps = []
    for c in range(NCORES):
        sl = slice(c * BSH, (c + 1) * BSH)
        in_maps.append({
            "feature": f3[sl],
            "tags": np.ascontiguousarray(tgf[sl]),
            "consts": consts,
        })
    return in_maps


def _run(in_maps, trace=False, tmpdir=None):
    from concourse.bass_utils import run_bass_kernel_spmd
    nc = _get_nc()
    res = run_bass_kernel_spmd(
        nc, in_maps, list(range(NCORES)), trace=trace, tmpdir=tmpdir
    )
    return res


def kernel(feature, tags, transitions):
    in_maps = _prep_inputs(feature, tags, transitions)
    res = _run(in_maps)
    total = np.float64(0.0)
    for c in range(NCORES):
        total += np.float64(res.results[c]["out"][0, 0])
    return np.float32(total)


# revision 49
# speedup vs baseline: 1.0755x; 1.0755x over previous
"""CRF NLL kernel for Trainium2 (8 NeuronCores, data-parallel over batch).

Self-contained: hardcodes shapes BS=8192, T=512, K=5.

Math: the 5-state CRF collapses to 3 live states {B,I,O} (START row and
STOP column of exp(transitions) are exactly 0).  The forward algorithm
runs in exp space.  Two time steps are fused into one "superstep":

    a_{2s+2} = W2_s @ a_{2s},   W2_s = D_{2s+2} (E3 D_{2s+1} E3)

The 3x3 W2_s matrices are built in bulk on the GPSIMD engine (E3 has 3
structurally-zero entries from the masked transitions, so E D E needs
only 12 nonzero multiply-accumulate terms, expressed as 7 regular-AP
ops), while the vector engine runs the serial chain at 2 ops per
superstep (multiply + segmented reduce).  Max-normalization every 6
supersteps; norm factors are log'd in bulk at the end.

Gold path score: per-partition accumulators via compare ops
(trans: 9-bin counts of idx=3*tag_t+tag_{t-1} dotted with the 3x3
table; emit: (tag==k)*f_k sweeps; plus START/STOP boundary terms).

Data parallel: batch 8192 -> 8 cores x 1024; per core 1024 = 8 groups
x 128 partitions.  Per-core partials summed on the host.
"""

import numpy as np
from contextlib import ExitStack

BS, T, K = 8192, 512, 5
NCORES = 8
BSH = BS // NCORES      # 1024 batch per core
G = BSH // 128          # 8 groups
START, STOP = 3, 4
S = (T - 1) // 2        # 255 supersteps covering t=1..510; t=511 leftover
QS = 9                  # supersteps between max-normalizations
NE = len([s for s in range(S) if s % QS == QS - 1])

# time chunks aligned to superstep boundaries (odd width => superstep
# pairs (t_odd=2s+1, t_even=2s+2) never straddle a boundary).  First
# chunks are small so the DMA->exp->W2-build->serial pipeline fills fast.
CH_TW = [33, 32, 64, 64, 64, 64, 64, 64, 63]
NCH = len(CH_TW)
CH_T0 = [sum(CH_TW[:c]) for c in range(NCH)]
assert sum(CH_TW) == T
# supersteps s with both 2s+1 and 2s+2 in [t0, t0+tw)
CH_S0 = [CH_T0[c] // 2 for c in range(NCH)]
CH_SN = [
    (CH_T0[c] + CH_TW[c] - 3) // 2 - CH_S0[c] + 1 for c in range(NCH)
]
assert CH_S0[-1] + CH_SN[-1] == S
assert all(CH_S0[c] + CH_SN[c] == CH_S0[c + 1] for c in range(NCH - 1))

_cache = {}


def _build():
    import concourse.bacc as bacc
    import concourse.mybir as mybir
    from concourse.tile import TileContext
    from concourse.alu_op_type import AluOpType as op
    AF = mybir.ActivationFunctionType
    f32 = mybir.dt.float32
    bf16 = mybir.dt.bfloat16
    AX = mybir.AxisListType

    nc = bacc.Bacc(
        "TRN2", target_bir_lowering=False, debug=False, num_devices=NCORES
    )
    feat_p = nc.declare_dram_parameter("feature", [BSH, 3, T], bf16, isOutput=False)
    tags_p = nc.declare_dram_parameter("tags", [BSH, T], bf16, isOutput=False)
    cst_p = nc.declare_dram_parameter("consts", [128, 64], f32, isOutput=False)
    out_p = nc.declare_dram_parameter("out", [1, 4], f32, isOutput=True)

    featr = feat_p[:].rearrange("(g p) k t -> p k g t", p=128)
    tagsr = tags_p[:].rearrange("(g p) t -> p g t", p=128)

    # consts columns:
    #  0-8  E3[j,i] (j-major)         9-11 E5[0:3,START]
    # 12-14 E5[STOP,0:3]              15   ones
    # 16-24 tr3 flat                  25-27 tr[k,START]
    # 28-30 tr[STOP,k]
    # 32-37 M2 (k=2: j in {0,2} x i)  38-39 M1a (j=1, i in {0,1})
    # 40-41 M1b (j=2, i in {0,1})     42 M0a (j=1,i=2)   43 M0b (j=2,i=2)
    with TileContext(nc) as tc, ExitStack() as ctx:
        sb = ctx.enter_context(tc.tile_pool(name="sb", bufs=1))
        ps = ctx.enter_context(tc.tile_pool(name="ps", bufs=1, space="PSUM"))

        cst = sb.tile([128, 64], f32)
        tagsf = sb.tile([128, G, T], bf16)
        fbufs = []
        dodds = []
        devens = []
        w2s = []
        for c in range(NCH):
            tw = CH_TW[c]
            sn = CH_SN[c]
            fb = sb.tile([128, 3, G, tw], bf16, name=f"fb{c}")
            nc.sync.dma_start(
                out=fb[:], in_=featr[:, :, :, CH_T0[c] : CH_T0[c] + tw]
            )
            fbufs.append(fb)
            if c == 0:
                # small, needed by the first W2 build / init
                nc.sync.dma_start(out=cst[:], in_=cst_p[:])
            if c == 1:
                # tags (1 MB bf16): land early enough that the mask ops the
                # scheduler slots mid-serial never wait on them
                nc.sync.dma_start(out=tagsf[:], in_=tagsr[:])
            # exp of odd/even steps packed [p, s, g, 3] so (s,g) merges
            # into one AP dim for the 3D-limited STT build ops
            o_odd = 2 * CH_S0[c] + 1 - CH_T0[c]
            do = sb.tile([128, sn, G, 3], f32, name=f"do{c}")
            de = sb.tile([128, sn, G, 3], f32, name=f"de{c}")
            nc.scalar.activation(
                do[:],
                fb[:, :, :, o_odd : o_odd + 2 * sn - 1 : 2]
                .rearrange("p k g s -> p s g k"),
                AF.Exp,
            )
            nc.scalar.activation(
                de[:],
                fb[:, :, :, o_odd + 1 : o_odd + 2 * sn : 2]
                .rearrange("p k g s -> p s g k"),
                AF.Exp,
            )
            dodds.append(do)
            devens.append(de)
            w2 = sb.tile([128, sn, G, 9], bf16, name=f"w2{c}")
            w2s.append(w2)
        # d for the init step t=0 and the leftover step t=511
        dinit = sb.tile([128, G, 3], f32)
        dlast = sb.tile([128, G, 3], f32)
        nc.scalar.activation(
            dinit[:], fbufs[0][:, :, :, 0].rearrange("p k g -> p g k"), AF.Exp
        )
        nc.scalar.activation(
            dlast[:],
            fbufs[-1][:, :, :, T - 1 - CH_T0[-1]].rearrange("p k g -> p g k"),
            AF.Exp,
        )

        e3 = cst[:, 0:9].rearrange("p (j i) -> p j i", j=3)
        ecol = cst[:, 9:12]
        estop = cst[:, 12:15]
        ones = cst[:, 15:16]
        trf = cst[:, 16:25]

        # ---- bulk W2 build on GPSIMD ----
        # G2 = E3 D_odd E3 with E3 zeros at (0,0),(0,1),(1,2):
        #  k=2 -> entries {0,2}x{0,1,2}; k=1 -> {1,2}x{0,1}; k=0 -> {1,2}x{2}
        # overlaps: (2,0),(2,1) [k1+k2], (2,2) [k0+k2]
        # All APs 3D (walrus limit): (s,g) merged into one dim N = sn*G.
        tmpA = sb.tile([128, 32 * G, 2], f32)
        tmpB = sb.tile([128, 32 * G, 1], f32)
        def _build_w2(eng, w2F, doF, deF, n0, n1):
            N = n1 - n0
            w2f = w2F[:, n0:n1, :]
            dof = doF[:, n0:n1, :]
            def_ = deF[:, n0:n1, :]

            def dk(k):
                return dof[:, :, k : k + 1].broadcast_to((128, N, 3))

            def mc(c0, n):
                return (
                    cst[:, c0 : c0 + n].unsqueeze(1).broadcast_to((128, N, n))
                )

            # j=0 row <- d2 * M2[j0]
            eng.tensor_tensor(w2f[:, :, 0:3], dk(2), mc(32, 3), op.mult)
            # j=2 row <- d2 * M2[j2]
            eng.tensor_tensor(w2f[:, :, 6:9], dk(2), mc(35, 3), op.mult)
            # (1,0),(1,1) <- d1 * M1a
            eng.tensor_tensor(
                w2f[:, :, 3:5], dk(1)[:, :, 0:2], mc(38, 2), op.mult
            )
            # (2,0),(2,1) += d1 * M1b
            eng.tensor_tensor(
                tmpA[:, :N, :], dk(1)[:, :, 0:2], mc(40, 2), op.mult
            )
            eng.tensor_tensor(
                w2f[:, :, 6:8], w2f[:, :, 6:8], tmpA[:, :N, :], op.add
            )
            # (1,2) <- d0 * M0a
            eng.tensor_tensor(
                w2f[:, :, 5:6], dk(0)[:, :, 0:1], mc(42, 1), op.mult
            )
            # (2,2) += d0 * M0b
            eng.tensor_tensor(
                tmpB[:, :N, :], dk(0)[:, :, 0:1], mc(43, 1), op.mult
            )
            eng.tensor_tensor(
                w2f[:, :, 8:9], w2f[:, :, 8:9], tmpB[:, :N, :], op.add
            )
            # fold outer diag, one op per j-row: W2[j,:] *= d_even[j]
            for j in range(3):
                lb = eng.tensor_tensor(
                    w2f[:, :, 3 * j : 3 * j + 3],
                    w2f[:, :, 3 * j : 3 * j + 3],
                    def_[:, :, j : j + 1].broadcast_to((128, N, 3)),
                    op.mult,
                )
            return lb

        for c in range(NCH):
            do, de, w2 = dodds[c], devens[c], w2s[c]
            sn = CH_SN[c]
            w2F = w2[:].rearrange("p s g e -> p (s g) e")
            doF = do[:].rearrange("p s g k -> p (s g) k")
            deF = de[:].rearrange("p s g k -> p (s g) k")
            # chunk 0 built in two s-halves so the serial chain's first
            # supersteps get their W2 as early as possible
            if c == 0:
                half = (sn // 2) * G
                _build_w2(nc.gpsimd, w2F, doF, deF, 0, half)
                last_build = _build_w2(nc.gpsimd, w2F, doF, deF, half, sn * G)
            else:
                last_build = _build_w2(nc.gpsimd, w2F, doF, deF, 0, sn * G)

        # ---- W4 pair-combine for the last chunks on Pool ----
        # W4_u = W2_{2u+1} @ W2_{2u}; looped over output row j so every AP
        # stays at 3 free dims (Pool TT legality).  Halves the serial cost
        # of these chunks.
        W4CH = (7, 8)
        w4s = {}
        tmpC = sb.tile([128, 16, G, 3], bf16)
        for c in W4CH:
            sn = CH_SN[c]
            npair = sn // 2
            w2t = w2s[c]
            w4 = sb.tile([128, npair, G, 9], bf16, name=f"w4{c}")
            w4s[c] = w4
            for j in range(3):
                for k in range(3):
                    b_ = (
                        w2t[:, 1 : 2 * npair : 2, :, 3 * j + k]
                        .unsqueeze(3).broadcast_to((128, npair, G, 3))
                    )
                    a_ = w2t[:, 0 : 2 * npair - 1 : 2, :, 3 * k : 3 * k + 3]
                    if k == 0:
                        nc.gpsimd.tensor_tensor(
                            w4[:, :, :, 3 * j : 3 * j + 3], b_, a_, op.mult
                        )
                    else:
                        nc.gpsimd.tensor_tensor(
                            tmpC[:, :npair, :, :], b_, a_, op.mult
                        )
                        last_build = nc.gpsimd.tensor_tensor(
                            w4[:, :, :, 3 * j : 3 * j + 3],
                            w4[:, :, :, 3 * j : 3 * j + 3],
                            tmpC[:, :npair, :, :], op.add,
                        )

        # ---------------- serial chain ----------------
        a = sb.tile([128, G, 3], f32)
        tmp = sb.tile([128, G, 3, 3], f32)
        r = sb.tile([128, G], f32)
        mbuf = sb.tile([128, 32, G], f32)

        e3b = e3.unsqueeze(1).broadcast_to((128, G, 3, 3))
        ecolb = ecol.unsqueeze(1).broadcast_to((128, G, 3))
        estopb = estop.unsqueeze(1).broadcast_to((128, G, 3))

        # init: a = E5[:,START] * d_0
        nc.vector.tensor_tensor(a[:], dinit[:], ecolb, op.mult)

        state = {"ev": 0, "st": 0}

        def unit(wsl, nsteps):
            ab = a[:].unsqueeze(2).broadcast_to((128, G, 3, 3))
            nc.vector.tensor_tensor(tmp[:], ab, wsl, op.mult)
            nc.vector.tensor_reduce(a[:], tmp[:], axis=AX.X, op=op.add)
            state["st"] += nsteps
            if state["st"] >= 2 * QS:
                ev = state["ev"]
                nc.vector.tensor_reduce(
                    mbuf[:, ev, :], a[:], axis=AX.X, op=op.max
                )
                nc.vector.reciprocal(r[:], mbuf[:, ev, :])
                rb = r[:].unsqueeze(2).broadcast_to((128, G, 3))
                nc.vector.tensor_tensor(a[:], a[:], rb, op.mult)
                state["ev"] += 1
                state["st"] = 0

        for c in range(NCH):
            sn = CH_SN[c]
            if c in W4CH:
                npair = sn // 2
                for u_ in range(npair):
                    unit(
                        w4s[c][:, u_, :, :]
                        .rearrange("p g (j i) -> p g j i", j=3),
                        4,
                    )
                if sn % 2:   # odd superstep left at the chunk end
                    unit(
                        w2s[c][:, sn - 1, :, :]
                        .rearrange("p g (j i) -> p g j i", j=3),
                        2,
                    )
            else:
                for ls in range(sn):
                    unit(
                        w2s[c][:, ls, :, :]
                        .rearrange("p g (j i) -> p g j i", j=3),
                        2,
                    )
        NEv = state["ev"]
        assert NEv <= 32

        # leftover step t = 511
        u = sb.tile([128, G, 3], f32)
        ab = a[:].unsqueeze(2).broadcast_to((128, G, 3, 3))
        nc.vector.tensor_tensor(tmp[:], ab, e3b, op.mult)
        nc.vector.tensor_reduce(u[:], tmp[:], axis=AX.X, op=op.add)
        nc.vector.tensor_tensor(a[:], u[:], dlast[:], op.mult)

        # terminal: fwd[p,g] = log(sum_j a[j]*estop[j]) + sum_e log(m[e])
        term = sb.tile([128, G], f32)
        flog = sb.tile([128, G], f32)
        last_serial = nc.vector.tensor_tensor(tmp[:, :, 0, :], a[:], estopb, op.mult)
        nc.vector.tensor_reduce(term[:], tmp[:, :, 0, :], axis=AX.X, op=op.add)
        nc.scalar.activation(flog[:], term[:], AF.Ln)

        mll = sb.tile([128, NEv, G], f32)
        mred = sb.tile([128, G], f32)
        nc.scalar.activation(mll[:], mbuf[:, :NEv, :], AF.Ln)
        mll_ge = mll[:].rearrange("p e g -> p g e")
        nc.vector.tensor_reduce(mred[:], mll_ge, axis=AX.X, op=op.add)

        ftot = sb.tile([128, G], f32)
        nc.vector.tensor_tensor(ftot[:], flog[:], mred[:], op.add)

        from concourse.bass import _add_dep_helper as add_dep

        # ---------------- gold path score ----------------
        idx = sb.tile([128, G, T - 1], bf16)
        junk = sb.tile([128, G * (T - 1)], bf16)
        junk2 = sb.tile([128, G, 65], bf16)
        cnts = sb.tile([128, 16], f32)
        eacc = sb.tile([128, NCH, 3], f32)

        # idx = 3*tag_t + tag_{t-1} on Pool (plain TTs; STT not Pool-legal)
        c3b = cst[:, 44:45].unsqueeze(1).broadcast_to((128, G, T - 1))
        ix0 = nc.gpsimd.tensor_tensor(
            idx[:], tagsf[:, :, 1:], c3b, op.mult
        )
        add_dep(ix0.ins, last_build.ins, reason="idx after W2 builds")
        idx_i = nc.gpsimd.tensor_tensor(
            idx[:], idx[:], tagsf[:, :, : T - 1], op.add
        )
        idxf = idx[:].rearrange("p g t -> p (g t)")
        for m in range(9):
            cnt_i = nc.vector.tensor_scalar(
                junk[:], idxf, float(m), None, op.is_equal, op.add,
                accum_out=cnts[:, m : m + 1],
            )
            add_dep(cnt_i.ins, last_serial.ins, reason="counts after serial chain")
        tmp9 = sb.tile([128, 9], f32)
        transp = sb.tile([128, 1], f32)
        nc.vector.tensor_tensor(tmp9[:, 0:8], cnts[:, 0:8], trf[:, 0:8], op.mult)
        nc.vector.tensor_reduce(transp[:], tmp9[:, 0:8], axis=AX.X, op=op.add)
        # + trf[8]*(cnt_8) with cnt_8 = (T-1)-sum(cnt_m): host stores
        # trf' = trf - trf8 in cols 16-23 and trf8*(T-1)*G... per-(p,g)
        # constant folded here per partition-row (G groups each):
        nc.vector.tensor_scalar(
            transp[:], transp[:], cst[:, 48:49], None, op.add
        )

        # emission gold: masks on DVE (is_equal not Pool-legal), the
        # mult/add accumulation on Pool, one final DVE reduce.
        maskb = sb.tile([128, 3, G, T], bf16)
        junk2b = sb.tile([128, G * T], bf16)
        prodb = sb.tile([128, G, T], bf16)
        for k in range(3):
            nc.vector.tensor_scalar(
                maskb[:, k], tagsf[:], float(k), None, op.is_equal
            )
            for c in range(NCH):
                tw = CH_TW[c]
                tsl = slice(CH_T0[c], CH_T0[c] + tw)
                msl = maskb[:, k, :, tsl]
                if k == 0:
                    p_i = nc.gpsimd.tensor_tensor(
                        prodb[:, :, tsl], msl, fbufs[c][:, 0, :, :], op.mult
                    )
                else:
                    p_i = nc.gpsimd.tensor_tensor(
                        junk2[:, :, :tw], msl, fbufs[c][:, k, :, :], op.mult
                    )
                    nc.gpsimd.tensor_tensor(
                        prodb[:, :, tsl], prodb[:, :, tsl], junk2[:, :, :tw],
                        op.add,
                    )
                add_dep(p_i.ins, last_build.ins, reason="emit after W2 builds")
        em_i = nc.vector.tensor_scalar(
            junk2b[:], prodb[:].rearrange("p g t -> p (g t)"), 0.0, None,
            op.add, op.add, accum_out=eacc[:, 0, 0:1],
        )
        add_dep(em_i.ins, last_serial.ins, reason="emit reduce after serial")

        # boundary: tr[tag_0, START] and tr[STOP, tag_{T-1}]
        bnd = sb.tile([128, 6, G], f32)
        for k in range(3):
            b_i = nc.vector.tensor_scalar(
                bnd[:, k, :], tagsf[:, :, 0], float(k), cst[:, 25 + k : 26 + k],
                op.is_equal, op.mult,
            )
            add_dep(b_i.ins, last_serial.ins, reason="gold tail after serial")
            b_i = nc.vector.tensor_scalar(
                bnd[:, 3 + k, :], tagsf[:, :, T - 1], float(k), cst[:, 28 + k : 29 + k],
                op.is_equal, op.mult,
            )
            add_dep(b_i.ins, last_serial.ins, reason="gold tail after serial")
        bred = sb.tile([128, G], f32)
        nc.vector.tensor_reduce(bred[:], bnd[:].rearrange("p s g -> p g s"),
                                axis=AX.X, op=op.add)

        # ---------------- combine ----------------
        nllg = sb.tile([128, G], f32)
        nc.vector.tensor_tensor(nllg[:], ftot[:], bred[:], op.subtract)
        red1 = sb.tile([128, 4], f32)
        nc.vector.tensor_reduce(red1[:, 0:1], nllg[:], axis=AX.X, op=op.add)
        nc.vector.tensor_copy(red1[:, 1:2], eacc[:, 0, 0:1])
        tot = sb.tile([128, 1], f32)
        nc.vector.tensor_tensor(tot[:], red1[:, 0:1], transp[:], op.subtract)
        nc.vector.tensor_tensor(tot[:], tot[:], red1[:, 1:2], op.subtract)

        acc = ps.tile([1, 1], f32)
        nc.tensor.matmul(acc[:], ones, tot[:], start=True, stop=True)
        osb = sb.tile([1, 4], f32)
        nc.vector.memset(osb[:], 0.0)
        nc.vector.tensor_copy(osb[:, 0:1], acc[:])
        nc.sync.dma_start(out=out_p[:], in_=osb[:])

    nc.compile()
    return nc


def _get_nc():
    if "nc" not in _cache:
        _cache["nc"] = _build()
    return _cache["nc"]


def _prep_inputs(feature, tags, transitions):
    f = np.asarray(feature, dtype=np.float32)
    tg = np.asarray(tags)
    tr = np.asarray(transitions, dtype=np.float32)

    E5 = np.exp(tr)
    E3 = E5[:3, :3]
    consts = np.zeros((128, 64), np.float32)
    consts[:, 0:9] = E3.reshape(-1)[None, :]
    consts[:, 9:12] = E5[:3, START][None, :]
    consts[:, 12:15] = E5[STOP, :3][None, :]
    consts[:, 15] = 1.0
    trf9 = tr[:3, :3].reshape(-1).astype(np.float64)
    consts[:, 16:24] = (trf9[:8] - trf9[8])[None, :].astype(np.float32)
    consts[:, 24] = 0.0
    consts[:, 48] = np.float32(G * trf9[8] * (T - 1))
    consts[:, 25:28] = tr[:3, START][None, :]
    consts[:, 28:31] = tr[STOP, :3][None, :]
    # M tables for the sparse E3 D E3 build
    consts[:, 32:38] = np.array(
        [E3[0, 2] * E3[2, 0], E3[0, 2] * E3[2, 1], E3[0, 2] * E3[2, 2],
         E3[2, 2] * E3[2, 0], E3[2, 2] * E3[2, 1], E3[2, 2] * E3[2, 2]],
        np.float32)[None, :]
    consts[:, 38:40] = np.array(
        [E3[1, 1] * E3[1, 0], E3[1, 1] * E3[1, 1]], np.float32)[None, :]
    consts[:, 40:42] = np.array(
        [E3[2, 1] * E3[1, 0], E3[2, 1] * E3[1, 1]], np.float32)[None, :]
    consts[:, 42] = E3[1, 0] * E3[0, 2]
    consts[:, 43] = E3[2, 0] * E3[0, 2]
    consts[:, 44] = 3.0
    consts[:, 45] = 0.0
    consts[:, 46] = 1.0
    consts[:, 47] = 2.0

    import ml_dtypes
    f3 = np.ascontiguousarray(f[:, :, :3].transpose(0, 2, 1)).astype(
        ml_dtypes.bfloat16
    )
    tgf = tg.astype(ml_dtypes.bfloat16)

    in_maps = []
    for c in range(NCORES):
        sl = slice(c * BSH, (c + 1) * BSH)
        in_maps.append({
            "feature": f3[sl],
            "tags": np.ascontiguousarray(tgf[sl]),
            "consts": consts,
        })
    return in_maps


def _run(in_maps, trace=False, tmpdir=None):
    from concourse.bass_utils import run_bass_kernel_spmd
    nc = _get_nc()
    res = run_bass_kernel_spmd(
        nc, in_maps, list(range(NCORES)), trace=trace, tmpdir=tmpdir
    )
    return res


def kernel(feature, tags, transitions):
    in_maps = _prep_inputs(feature, tags, transitions)
    res = _run(in_maps)
    total = np.float64(0.0)
    for c in range(NCORES):
        total += np.float64(res.results[c]["out"][0, 0])
    return np.float32(total)
